# revision 1
# baseline (speedup 1.0000x reference)
"""Trainium2 Bass kernel for nn_MultiHeadAttention_4999341933079.

Multi-head attention, B=8, N=1024, dim=768, 16 heads, head_dim=48, with the
torch-faithful raw-memory reshapes:
    qkv  = x @ Wqkv                      # [B, N, 2304]
    q,k,v = raw_view(qkv, (3, B, 16, N, 48))
    out  = softmax(q k^T / sqrt(48)) v   -> raw_view -> @ Wo + bo

Sharding: data parallel over the OUTPUT batch.  Core j owns output batch j.
Because the raw reshape scrambles batches, core j's Q/K/V head slabs are three
contiguous flat ranges of the qkv GEMM output.  Each range covers ~342 rows of
the [8192, 2304] GEMM, so core j only computes 3x384 = 1152 GEMM rows (1/8 of
the work, zero collectives, zero redundancy up to padding).

Per-core pipeline (one SPMD program on 8 cores):
  Ph1: qkv GEMM (bf16) -> scatter into DRAM scratch at a dynamic offset
       (core-dependent sub-row shift r_s = (2s + j) % 3, from partition_id):
         zq/zk: [rows, 128] bf16, Q^T/K^T source data duplicated at cols 0-47
                and 64-111 so one DMA-transpose load yields both row-group
                copies for concurrently row-tiled matmuls,
         zv:    [rows, 48] bf16 natural layout.
  Ph2: per head h: DMA-transpose loads Q^T,K^T [128,1024]; S^T = K^T.T @ Q^T
       row-tiled over two 48-row groups; exp via ScalarE (scale folded in, no
       max subtraction -- |S| <= ~6), the write AP permuting the query axis
       into output-token order; C^T = [V|ones].T @ expS^T col-tiled 2 heads
       per pass, the ones column giving softmax denominators for free.
  Per 4 heads (software-pipelined against later heads so PE never waits on
  the recip chain): recip = exp(-ln(sums)) on ScalarE; broadcast via a 0/1
  selection matmul (f32r); normalize C^T; scatter to concat^T (DRAM); and,
  one group later, the per-pair output GEMM  out = concat^T.T @ Wo + bo
  (output tokens 128u..128u+127 depend only on head pair u).

Accepts FULL inputs, returns FULL output.  Host work is only
slice/transpose/cast packing (no FLOPs).
"""

import numpy as np
import ml_dtypes

import concourse.mybir as mybir
import concourse.tile as tile
from concourse import bacc
from concourse import bass_utils
from concourse.bass import ds

B = 8
SEQ = 1024
DIM = 768
HEADS = 16
DH = 48
SCALE = DH ** -0.5
C3 = 3 * DIM          # 2304
RROWS = 384           # packed x rows per range (>= 342 actually needed)
TROWS = 3 * RROWS     # 1152
ZBUF = RROWS * 48 + 32  # z-rows per range + shift headroom

BF16 = mybir.dt.bfloat16
F32 = mybir.dt.float32

# qkv GEMM free-dim chunks: 48-aligned (for the scatter copies) and <= 512
# (PSUM bank limit).
CHUNKS = [(0, 480), (480, 480), (960, 480), (1440, 480), (1920, 384)]

TRACE = False
TRACE_ALL_CORES = True
LAST_EXEC_NS = None
LAST_RESULTS = None


def _kernel_body(nc, tc, xT, wq, wo, bo, sel, out, zq, zk, zv, concd):
    Exp = mybir.ActivationFunctionType.Exp
    Log = mybir.ActivationFunctionType.Ln
    F32R = mybir.dt.float32r

    # Core-dependent sub-row shift r_s = (2s + core) % 3 applied as a
    # dynamic DRAM offset on the (gpsimd-issued) z-buffer stores.
    pid = nc.gpsimd.partition_id()
    shift_rows = [32 - 16 * ((2 * s + pid) % 3) for s in range(3)]

    with tc.tile_pool(name="const", bufs=1) as constp, \
         tc.tile_pool(name="work", bufs=2) as work, \
         tc.tile_pool(name="pexp", bufs=2) as pexp, \
         tc.tile_pool(name="pct", bufs=4) as pct, \
         tc.tile_pool(name="psall", bufs=4, space="PSUM") as psall:
        wo_sb = constp.tile([128, 6, DIM], BF16)
        nc.sync.dma_start(wo_sb, wo.rearrange("(q p) f -> p q f", p=128))
        bo_sb = constp.tile([128, DIM], F32)
        nc.sync.dma_start(bo_sb, bo)
        # head h sums/recip live at partition h%4 + 32*(h//4) (norm groups
        # of 4 heads; engine ops need 32-aligned partition bases)
        sums_all = constp.tile([128, SEQ], F32)
        sel_sb = constp.tile([128, 8 * 128], F32R)
        nc.sync.dma_start(sel_sb, sel)
        recip_all = constp.tile([128, SEQ], F32R)
        nc.vector.memset(recip_all.bitcast(mybir.dt.uint32), 0)

        xt_sb = constp.tile([128, 6, TROWS], BF16)
        wq_sb = constp.tile([128, 6, C3], BF16)
        for k in range(6):
            nc.sync.dma_start(
                xt_sb[:, k, :],
                xT.rearrange("(q p) t -> p q t", p=128)[:, k, :])
            nc.sync.dma_start(
                wq_sb[:, k, :],
                wq.rearrange("(q p) c -> p q c", p=128)[:, k, :])

        # ---------- Phase 1: qkv GEMM + scatter to z buffers ----------
        # Manually-rotated staging buffers; pad cols initialized once.
        stgs = [constp.tile([128, 48, 128], BF16, name=f"stg{i}")
                for i in range(2)]
        for t in stgs:
            nc.vector.memset(t[:, :, 48:64], 0.0)
            nc.vector.memset(t[:, :, 112:128], 0.0)

        CPAIRS = [(0, 1), (2, 3), (4,)]
        for s in range(3):
            for ttile in range(3):
                t0 = s * RROWS + ttile * 128
                psc = {}
                for pi, pair in enumerate(CPAIRS):
                    for c in pair:
                        psc[c] = psall.tile([128, 10, DH], F32, tag="ps",
                                            name=f"psc_{s}_{ttile}_{c}")
                    for k in range(6):
                        lhsT = xt_sb[:, k, t0:t0 + 128]
                        for c in pair:
                            c0, cw = CHUNKS[c]
                            nj = cw // DH
                            nc.tensor.matmul(
                                psc[c][:, :nj, :], lhsT,
                                wq_sb[:, k, c0:c0 + cw],
                                start=(k == 0), stop=(k == 5))
                if s < 2:
                    zdst = zq if s == 0 else zk
                    stg = stgs[(s * 3 + ttile) % 2]
                    for c, (c0, cw) in enumerate(CHUNKS):
                        j0, nj = c0 // DH, cw // DH
                        nc.vector.tensor_copy(
                            out=stg[:, j0:j0 + nj, 0:DH],
                            in_=psc[c][:, :nj, :])
                        nc.vector.tensor_copy(
                            out=stg[:, j0:j0 + nj, 64:64 + DH],
                            in_=psc[c][:, :nj, :])
                    nc.gpsimd.dma_start(
                        zdst[ds(shift_rows[s] + 6144 * ttile, 6144), :], stg)
                else:
                    stgv = work.tile([128, 48, DH], BF16, tag="stgv")
                    for c, (c0, cw) in enumerate(CHUNKS):
                        j0, nj = c0 // DH, cw // DH
                        nc.vector.tensor_copy(
                            out=stgv[:, j0:j0 + nj, :],
                            in_=psc[c][:, :nj, :])
                    nc.gpsimd.dma_start(
                        zv[ds(shift_rows[2] + 6144 * ttile, 6144), :], stgv)

        # ---------- Phase 2: attention per head ----------
        # The exp writes permute the query axis n' -> n'' = 64*(n'%16) + n'//16
        # so that C^T columns land in output-token-friendly order: the final
        # raw reshape sends ctx[h, n', d] to out token n = 64h + n'//16,
        # feature col 48*(n'%16) + d.

        def srow(h):
            return h % 4 + 32 * (h // 4)

        def ogemm_pair(pu, piece):
            pso = psall.tile([128, DIM], F32, tag="ps", name=f"pso_{pu}")
            for q in range(6):
                for c0, cw in ((0, 512), (512, 256)):
                    nc.tensor.matmul(
                        pso[:, c0:c0 + cw], piece[:, q, :],
                        wo_sb[:, q, c0:c0 + cw],
                        start=(q == 0), stop=(q == 5))
            outt = work.tile([128, DIM], F32, tag="outt", name=f"outt_{pu}")
            nc.vector.tensor_add(out=outt, in0=pso, in1=bo_sb)
            nc.sync.dma_start(out[128 * pu:128 * pu + 128, :], outt)

        def norm_group(grp, cts, prev_pieces):
            """recip for heads [4*grp, 4*grp+4), normalize+store; the
            output GEMM of the PREVIOUS group runs first (its data is
            ready, keeping PE busy while the recip chain completes)."""
            p0g = 32 * grp
            lnt = work.tile([4, SEQ], F32, tag="lnt", name=f"lnt{grp}")
            nc.scalar.activation(out=lnt, in_=sums_all[p0g:p0g + 4, :],
                                 func=Log)
            nc.scalar.activation(out=recip_all[p0g:p0g + 4, :], in_=lnt,
                                 func=Exp, scale=-1.0)
            for ppu, ppiece in prev_pieces:
                ogemm_pair(ppu, ppiece)
            pieces = []
            for u, ct in enumerate(cts):
                pu = 2 * grp + u
                psB = psall.tile([128, 2, 512], F32, tag="ps",
                                 name=f"psB_{grp}_{u}")
                for ci in range(2):
                    nc.tensor.matmul(
                        psB[0:112, ci, :],
                        sel_sb[:, 128 * pu:128 * pu + 112],
                        recip_all[:, ci * 512:(ci + 1) * 512],
                        start=True, stop=True)
                ctn = pct.tile([128, 4, 512], BF16, tag="ctn", bufs=2,
                               name=f"ctn_{grp}_{u}")
                nc.vector.tensor_mul(out=ctn[0:DH, 0:2, :],
                                     in0=ct[0:DH, 0:2, :],
                                     in1=psB[0:DH, 0:2, :])
                nc.vector.tensor_mul(out=ctn[64:64 + DH, 2:4, :],
                                     in0=ct[64:64 + DH, 2:4, :],
                                     in1=psB[64:64 + DH, 0:2, :])
                for hb, plo in ((0, 0), (1, 64)):
                    hh = 2 * pu + hb
                    csrc = ctn[plo:plo + DH, 2 * hb:2 * hb + 2, :] \
                        .rearrange("p c w -> p (c w)") \
                        .rearrange("p (j nn) -> p j nn", nn=64)
                    cdst = concd[:, 64 * hh:64 * hh + 64] \
                        .rearrange("(j d) nn -> d j nn", d=DH)
                    nc.sync.dma_start(cdst, csrc)
                # Output tokens [128*pu, 128*pu+128) depend only on this
                # head pair; load its concat^T piece, GEMM deferred to the
                # next norm group.
                piece = work.tile([128, 6, 128], BF16, tag="cpiece",
                                  name=f"piece_{pu}")
                nc.sync.dma_start(
                    piece,
                    concd[:, 128 * pu:128 * pu + 128]
                    .rearrange("(q p) n -> p q n", p=128))
                pieces.append((pu, piece))
            return pieces

        cts = []
        pending = None
        pieces = []
        exps_prev = None
        vt_prev = None
        for h in range(HEADS):
            r0 = 32 + SEQ * h
            qt = work.tile([128, SEQ], BF16, tag="qt")
            nc.sync.dma_start(qt, zq[r0:r0 + SEQ, :], transpose=True)
            kt = work.tile([128, SEQ], BF16, tag="kt")
            nc.sync.dma_start(kt, zk[r0:r0 + SEQ, :], transpose=True)

            # S^T: m-tiles (r, r+4) concurrently in row groups 0/64,
            # separate psum tiles; exp per half (n=1024).
            exps = pexp.tile([128, 4, 2, SEQ], BF16, tag="exps")
            for r in range(4):
                for half in range(2):
                    p0 = 64 * half
                    m = half * 4 + r
                    ps = psall.tile([128, 2, 512], F32, tag="ps",
                                    name=f"ps_{h}_{r}_{half}")
                    lhsT = kt[p0:p0 + DH, m * 128:(m + 1) * 128]
                    for ci in range(2):
                        nc.tensor.matmul(
                            ps[:, ci, :], lhsT,
                            qt[p0:p0 + DH, ci * 512:(ci + 1) * 512],
                            start=True, stop=True,
                            tile_position=(p0, 0))
                    # out AP permutes n' -> n'' on write; both APs iterate
                    # (nn outer, j inner) == linear n'.
                    eout = exps[:, r, half, :] \
                        .rearrange("p (j nn) -> p nn j", nn=64)
                    ein = ps.rearrange("p c w -> p (c w)") \
                        .rearrange("p (nn j) -> p nn j", j=16)
                    nc.scalar.activation(out=eout, in_=ein,
                                         func=Exp, scale=SCALE)

            vt = work.tile([128, 8, DH + 1], BF16, tag="vt")
            nc.vector.memset(vt[:, :, DH:DH + 1], 1.0)
            nc.gpsimd.dma_start(
                vt[:, :, 0:DH],
                zv[r0:r0 + SEQ, :].rearrange("(i p) d -> p i d", p=128))

            if h % 2 == 0:
                exps_prev, vt_prev = exps, vt
                continue

            # C^T for the head pair (h-1, h), col-tiled; ones col gives sums.
            ct = pct.tile([128, 4, 512], BF16, tag="ct", name=f"ct_{h // 2}")
            sums_sb = work.tile([128, 2, 512], F32, tag="sums_sb",
                                name=f"sums_sb_{h // 2}")
            for ci in range(2):
                psC = psall.tile([128, 2, 512], F32, tag="ps",
                                 name=f"psC_{h // 2}_{ci}")
                for i in range(8):
                    rr, hf = i % 4, i // 4
                    nc.tensor.matmul(
                        psC[0:DH + 1, 0, :], vt_prev[:, i, :],
                        exps_prev[:, rr, hf, ci * 512:(ci + 1) * 512],
                        start=(i == 0), stop=(i == 7))
                    nc.tensor.matmul(
                        psC[64:64 + DH + 1, 1, :], vt[:, i, :],
                        exps[:, rr, hf, ci * 512:(ci + 1) * 512],
                        start=(i == 0), stop=(i == 7),
                        tile_position=(0, 64))
                nc.vector.tensor_copy(out=ct[0:DH, ci, :],
                                      in_=psC[0:DH, 0, :])
                nc.vector.tensor_copy(out=ct[64:64 + DH, 2 + ci, :],
                                      in_=psC[64:64 + DH, 1, :])
                nc.vector.tensor_copy(out=sums_sb[32:DH + 1, ci, :],
                                      in_=psC[32:DH + 1, 0, :])
                nc.vector.tensor_copy(out=sums_sb[96:64 + DH + 1, ci, :],
                                      in_=psC[96:64 + DH + 1, 1, :])
            srA, srB = srow(h - 1), srow(h)
            nc.gpsimd.dma_start(sums_all[srA:srA + 1, :],
                                sums_sb[DH:DH + 1, :, :])
            nc.gpsimd.dma_start(sums_all[srB:srB + 1, :],
                                sums_sb[64 + DH:64 + DH + 1, :, :])
            cts.append(ct)
            if h % 4 == 1 and h >= 5:
                pieces = norm_group(h // 4 - 1, pending, pieces)
            if h % 4 == 3:
                pending = cts
                cts = []

        # final norm group + OGEMM flush
        pieces = norm_group(3, pending, pieces)
        for ppu, ppiece in pieces:
            ogemm_pair(ppu, ppiece)


def build_nc():
    nc = bacc.Bacc("TRN2", target_bir_lowering=False, debug=False,
                   num_devices=B)
    xT = nc.dram_tensor("xT", [DIM, TROWS], BF16, kind="ExternalInput").ap()
    wq = nc.dram_tensor("wqkv", [DIM, C3], BF16, kind="ExternalInput").ap()
    wo = nc.dram_tensor("wo", [DIM, DIM], BF16, kind="ExternalInput").ap()
    bo = nc.dram_tensor("bo_b", [128, DIM], F32, kind="ExternalInput").ap()
    sel = nc.dram_tensor("sel", [128, 8 * 128],
                         mybir.dt.float32r, kind="ExternalInput").ap()
    out = nc.dram_tensor("out", [SEQ, DIM], F32, kind="ExternalOutput").ap()
    zq = nc.dram_tensor("zq", [ZBUF, 128], BF16).ap()
    zk = nc.dram_tensor("zk", [ZBUF, 128], BF16).ap()
    zv = nc.dram_tensor("zv", [ZBUF, DH], BF16).ap()
    concd = nc.dram_tensor("concd", [DIM, SEQ], BF16).ap()

    with tile.TileContext(nc) as tc:
        _kernel_body(nc, tc, xT, wq, wo, bo, sel, out, zq, zk, zv, concd)
    nc.compile()
    return nc


_NC_CACHE = None


def _get_nc():
    global _NC_CACHE
    if _NC_CACHE is None:
        _NC_CACHE = build_nc()
    return _NC_CACHE


def _t0(s, j):
    # first qkv-GEMM row of core j's range s
    return ((s * 128 + 16 * j) * 64) // 3


def make_in_maps(x, Wqkv, Wo, bo):
    x_flat = np.asarray(x, np.float32).reshape(B * SEQ, DIM)
    wq_bf = np.asarray(Wqkv, np.float32).astype(ml_dtypes.bfloat16)
    wo_bf = np.asarray(Wo, np.float32).astype(ml_dtypes.bfloat16)
    bo_b = np.ascontiguousarray(
        np.broadcast_to(np.asarray(bo, np.float32)[None, :], (128, DIM)))
    # sel[:, 128*pair + p] selects recip rows for a head pair:
    # partitions 0-47 <- head 2*pair, partitions 64-111 <- head 2*pair+1.
    sel = np.zeros((128, 8 * 128), np.float32)
    for pu in range(8):
        hA, hB = 2 * pu, 2 * pu + 1
        rA = hA % 4 + 32 * (hA // 4)
        rB = hB % 4 + 32 * (hB // 4)
        sel[rA, 128 * pu + 0:128 * pu + DH] = 1.0
        sel[rB, 128 * pu + 64:128 * pu + 64 + DH] = 1.0
    in_maps = []
    for j in range(B):
        rows = np.zeros((TROWS, DIM), np.float32)
        for s in range(3):
            t0 = _t0(s, j)
            t1 = min(t0 + RROWS, B * SEQ)
            rows[s * RROWS: s * RROWS + (t1 - t0)] = x_flat[t0:t1]
        xT = np.ascontiguousarray(rows.T.astype(ml_dtypes.bfloat16))
        in_maps.append({"xT": xT, "wqkv": wq_bf, "wo": wo_bf, "bo_b": bo_b, "sel": sel})
    return in_maps


def kernel(x, Wqkv, Wo, bo):
    global LAST_EXEC_NS, LAST_RESULTS
    nc = _get_nc()
    in_maps = make_in_maps(x, Wqkv, Wo, bo)
    kwargs = {}
    if TRACE:
        kwargs = dict(trace=True,
                      trace_cores=list(range(B)) if TRACE_ALL_CORES else [0])
    res = bass_utils.run_bass_kernel_spmd(
        nc, in_maps, core_ids=list(range(B)), **kwargs)
    LAST_EXEC_NS = res.exec_time_ns
    LAST_RESULTS = res
    out = np.stack([res.results[j]["out"] for j in range(B)], axis=0)
    return np.ascontiguousarray(out.astype(np.float32))

